# revision 23
# baseline (speedup 1.0000x reference)
"""Batched normalized-gram kernel for 8 TRN2 NeuronCores.

reference:  x (64, 2, 512, 512) fp32
    x0 = x[:, 0]                               (B=64, V=512, F=512)
    n  = sqrt(sum(x0^2, axis=(0, 2)))          (V,)
    out[b] = (x0[b] @ x0[b].T) / outer(n, n)   (B, V, V)

gram[b,i,j]/(n_i n_j) == (x0[b,i,:]/n_i) . (x0[b,j,:]/n_j), so the host
prescales rows by 1/n once and the device work is a pure batched symmetric
matmul out[b] = y[b] @ y[b].T.

Device-side structure (per core, 8 batches; HBM-bound "ridge": 832 KB
of HBM traffic per batch ~ 2.3 us vs 2.16 us of PE streaming):
  * operands shipped as fp16 — halves input DMA, full-rate PE, fp32 PSUM
    accumulation keeps rel err ~3e-4.
  * upper block-triangle only (row-block mi covers columns mi*128..511);
    host mirrors the lower blocks.  -37.5% output DMA / PE work.
  * ONE input DMA per batch (HWDGE trigger costs ~0.65 us of descriptor
    generation, so more/smaller DMAs throttle the input stream): host
    packs y[b].T into [128, 4*512] so the 512 KB transfer is contiguous.
  * all 8 input batches buffered in SBUF up front — input prefetch never
    waits on compute.
  * fp16 output, segments packed mi0|mi1|mi3|mi2 so every segment's
    matmul accumulates inside a single 2 KiB PSUM bank.  Three separate
    PSUM tiles (bank0: mi0, bank1: mi1+mi3, bank2: mi2) give the
    PSUM->SBUF copies per-group dependencies, and the output leaves in
    two contiguous DMAs (512 cols after the Scalar copy, 768 cols after
    the Vector copies) to shorten the drain.
  * a zero-matmul warmup burst (4xN512 + 10xN128) fills the
    preamble->first-data window so the PE HAM clock-gate is at full
    rate when real matmuls start.

Sharding: data-parallel over batch — 8 batches per core, no collectives.
"""

import numpy as np

B, T, V, F = 64, 2, 512, 512
NCORES = 8
BPC = B // NCORES  # batches per core
NBLK = V // 128  # 4 row-blocks

# packed segment layout: row-block mi -> offset; order mi0|mi1|mi3|mi2
SEG_OFF = {0: 0, 1: 512, 3: 896, 2: 1024}
PACK = 1280
SPLIT = 512  # output part A = cols 0:512, part B = cols 512:1280

_NC = None


def _build_nc():
    import concourse.mybir as mybir
    import concourse.tile as tile
    from concourse import bacc

    f32 = mybir.dt.float32
    f16 = mybir.dt.float16
    COPY = mybir.ActivationFunctionType.Copy

    nc = bacc.Bacc(target_bir_lowering=False)
    yh = nc.declare_dram_parameter("yh", [128, V], f16, isOutput=False)
    yr = nc.declare_dram_parameter("yr", [128, 3 * V], f16, isOutput=False)
    yin = nc.declare_dram_parameter(
        "yin", [BPC - 1, 128, NBLK * V], f16, isOutput=False
    )
    outA = nc.declare_dram_parameter("outA", [BPC, 128, SPLIT], f16, isOutput=True)
    outB = nc.declare_dram_parameter(
        "outB", [BPC, 128, PACK - SPLIT], f16, isOutput=True
    )

    with tile.TileContext(nc) as tc:
        with (
            tc.tile_pool(name="inp", bufs=BPC) as inp_pool,
            tc.tile_pool(name="warm", bufs=1) as warm_pool,
            tc.tile_pool(name="ps0", bufs=2, space="PSUM") as ps0_pool,
            tc.tile_pool(name="ps1", bufs=2, space="PSUM") as ps1_pool,
            tc.tile_pool(name="ps2", bufs=2, space="PSUM") as ps2_pool,
            tc.tile_pool(name="ps3", bufs=1, space="PSUM") as ps3_pool,
            tc.tile_pool(name="psw", bufs=1, space="PSUM") as psw_pool,
            tc.tile_pool(name="outa", bufs=4) as outa_pool,
            tc.tile_pool(name="outb", bufs=4) as outb_pool,
        ):
            # batch 0's input lands as a 128 KB head (chunk 0) + 384 KB
            # rest — both contiguous DRAM regions — so its first matmuls
            # (ki=0) can start ~1 us earlier; later batches use one
            # 512 KB DMA each (every HWDGE trigger costs ~0.65 us of
            # descriptor generation, so fewer triggers = faster stream).
            h0 = inp_pool.tile([128, V], f16, tag="h0", bufs=1)
            nc.sync.dma_start(out=h0, in_=yh[:, :])
            h1 = inp_pool.tile([128, 3 * V], f16, tag="h1", bufs=1)
            nc.sync.dma_start(out=h1, in_=yr[:, :])
            tiles = [None]
            for b in range(1, BPC):
                it = inp_pool.tile([128, NBLK * V], f16, tag="in", bufs=7)
                nc.sync.dma_start(out=it, in_=yin[b - 1])
                tiles.append(it)

            # PE warmup on zeros: keep the PE busy from right after the
            # preamble until batch 0's data lands, ending with small-N
            # matmuls so batch 0 isn't delayed behind a long one.
            wz = warm_pool.tile([128, V], f16)
            nc.gpsimd.memset(wz, 0)
            wps = psw_pool.tile([128, V], f32)
            for _ in range(4):
                nc.tensor.matmul(wps, lhsT=wz[:, :128], rhs=wz, start=True, stop=True)
            for _ in range(4):
                nc.tensor.matmul(
                    wps[:, :128],
                    lhsT=wz[:, :128],
                    rhs=wz[:, :128],
                    start=True,
                    stop=True,
                )

            BANK = {0: 0, 1: 1, 3: 1, 2: 2}
            for b in range(BPC):
                p0 = ps0_pool.tile([128, 512], f32, tag="p0")
                p1 = ps1_pool.tile([128, 512], f32, tag="p1")
                p2 = ps2_pool.tile([128, 512], f32, tag="p2")
                ps = {0: p0, 1: p1, 2: p2}
                oa = outa_pool.tile([128, SPLIT], f16, tag="oa")
                ob = outb_pool.tile([128, PACK - SPLIT], f16, tag="ob")

                def seg_mm(mi, ki, src, col0):
                    bank = BANK[mi]
                    off = SEG_OFF[mi] - (0, 512, 1024)[bank]
                    n_cols = V - 128 * mi
                    nc.tensor.matmul(
                        ps[bank][:, off : off + n_cols],
                        lhsT=src[:, col0 + mi * 128 : col0 + mi * 128 + 128],
                        rhs=src[:, col0 + mi * 128 : col0 + V],
                        start=(ki == 0),
                        stop=(ki == NBLK - 1),
                    )

                def mi3_mm(ki, src, col0, p3):
                    # mi=3 into its own PSUM bank (for b0/b7 only)
                    nc.tensor.matmul(
                        p3[:, 0:128],
                        lhsT=src[:, col0 + 384 : col0 + 512],
                        rhs=src[:, col0 + 384 : col0 + V],
                        start=(ki == 0),
                        stop=(ki == NBLK - 1),
                    )

                if b == 0:
                    # ki-outer so the first matmuls need only the 128 KB
                    # head chunk while the rest is still in flight; mi3
                    # gets the spare PSUM bank so all FOUR groups can
                    # interleave (one open accumulation group per bank —
                    # a start=True matmul clears has_written for its
                    # whole bank), making the first round 4 matmuls.
                    p3 = ps3_pool.tile([128, 512], f32, tag="p3", bufs=1)
                    for ki in range(NBLK):
                        src = h0 if ki == 0 else h1
                        col0 = 0 if ki == 0 else (ki - 1) * V
                        for mi in (0, 1, 2):
                            seg_mm(mi, ki, src, col0)
                        mi3_mm(ki, src, col0, p3)
                    nc.scalar.activation(out=oa, in_=ps[0], func=COPY)
                    nc.sync.dma_start(out=outA[b], in_=oa)
                    nc.vector.tensor_copy(out=ob[:, 0:384], in_=ps[1][:, 0:384])
                    nc.vector.tensor_copy(out=ob[:, 384:512], in_=p3[:, 0:128])
                    nc.vector.tensor_copy(out=ob[:, 512:768], in_=ps[2][:, 0:256])
                    nc.sync.dma_start(out=outB[b], in_=ob)
                    continue

                if b == BPC - 1:
                    # last batch: mi3 in the spare bank and LAST, so the
                    # final matmul group, copy, and output piece are the
                    # small 128-column segment — shortest possible drain.
                    p3 = ps3_pool.tile([128, 512], f32, tag="p3", bufs=1)
                    for mi in (0, 1, 2):
                        for ki in range(NBLK):
                            seg_mm(mi, ki, tiles[b], ki * V)
                        if mi == 0:
                            nc.scalar.activation(out=oa, in_=ps[0], func=COPY)
                            nc.sync.dma_start(out=outA[b], in_=oa)
                        elif mi == 1:
                            nc.vector.tensor_copy(
                                out=ob[:, 0:384], in_=ps[1][:, 0:384]
                            )
                            nc.sync.dma_start(
                                out=outB[b, :, 0:384], in_=ob[:, 0:384]
                            )
                        else:
                            nc.vector.tensor_copy(
                                out=ob[:, 512:768], in_=ps[2][:, 0:256]
                            )
                            nc.sync.dma_start(
                                out=outB[b, :, 512:768], in_=ob[:, 512:768]
                            )
                    for ki in range(NBLK):
                        mi3_mm(ki, tiles[b], ki * V, p3)
                    nc.vector.tensor_copy(out=ob[:, 384:512], in_=p3[:, 0:128])
                    # scalar queue is idle here; avoids queuing the final
                    # 32 KB piece behind the sync triggers
                    nc.scalar.dma_start(
                        out=outB[b, :, 384:512], in_=ob[:, 384:512]
                    )
                    continue

                for mi in (0, 1, 3, 2):
                    for ki in range(NBLK):
                        seg_mm(mi, ki, tiles[b], ki * V)
                    if mi == 0:
                        nc.scalar.activation(out=oa, in_=ps[0], func=COPY)
                        nc.sync.dma_start(out=outA[b], in_=oa)
                    elif mi == 3:
                        nc.vector.tensor_copy(out=ob[:, 0:512], in_=ps[1])
                    elif mi == 2:
                        nc.vector.tensor_copy(
                            out=ob[:, 512:768], in_=ps[2][:, 0:256]
                        )
                        nc.sync.dma_start(out=outB[b], in_=ob)
    if not nc.is_finalized():
        nc.finalize()
    return nc


def _get_nc():
    global _NC
    if _NC is None:
        _NC = _build_nc()
    return _NC


def _prep_shards(x: np.ndarray) -> np.ndarray:
    x = np.ascontiguousarray(np.asarray(x, dtype=np.float32))
    x0 = x[:, 0]  # (B, V, F)
    ss = np.einsum("bvf,bvf->v", x0, x0, optimize=True)
    inv_n = (1.0 / np.sqrt(ss)).astype(np.float32)
    y = x0 * inv_n[None, :, None]
    # yT[b] is (F, V); lay out as [128, 4*V] with chunk k = rows k*128..
    # at columns k*V.. so each batch is one contiguous 512 KB DMA.
    yT = np.transpose(y, (0, 2, 1)).reshape(B, NBLK, 128, V)
    return np.ascontiguousarray(np.transpose(yT, (0, 2, 1, 3))).astype(
        np.float16
    ).reshape(B, 128, NBLK * V)


def kernel(x: np.ndarray, _trace: bool = False, _trace_out: list | None = None):
    from concourse.bass_utils import run_bass_kernel_spmd

    yin = _prep_shards(x)
    nc = _get_nc()
    in_maps = [
        {
            "yh": np.ascontiguousarray(yin[c * BPC, :, :V]),
            "yr": np.ascontiguousarray(yin[c * BPC, :, V:]),
            "yin": yin[c * BPC + 1 : (c + 1) * BPC],
        }
        for c in range(NCORES)
    ]
    res = run_bass_kernel_spmd(
        nc, in_maps, core_ids=list(range(NCORES)), trace=_trace
    )
    if _trace_out is not None:
        _trace_out.append(res)
    packedA = np.concatenate(
        [np.asarray(res.results[c]["outA"]) for c in range(NCORES)], axis=0
    )  # (B, 128, 512)
    packedB = np.concatenate(
        [np.asarray(res.results[c]["outB"]) for c in range(NCORES)], axis=0
    )  # (B, 128, 768)
    packed = np.concatenate([packedA, packedB], axis=2)
    full = np.empty((B, V, V), dtype=np.float32)
    for mi in range(NBLK):
        off = SEG_OFF[mi]
        n_cols = V - 128 * mi
        full[:, mi * 128 : (mi + 1) * 128, mi * 128 :] = packed[
            :, :, off : off + n_cols
        ]
    # device wrote only the upper block-triangle; mirror it down
    for mi in range(NBLK):
        for nj in range(mi + 1, NBLK):
            full[:, nj * 128 : (nj + 1) * 128, mi * 128 : (mi + 1) * 128] = (
                np.swapaxes(
                    full[:, mi * 128 : (mi + 1) * 128, nj * 128 : (nj + 1) * 128],
                    1,
                    2,
                )
            )
    return full
